# revision 3
# baseline (speedup 1.0000x reference)
"""MHSA Trainium2 Bass kernel (8 NeuronCores, SPMD).

Sharding: core c handles batch b=c//4, head group hg=c%4 (4 of 16 heads).
Device computes, per core: Q/K/V projections (column-sharded per head
group), head-local attention with softmax(sigmoid(s)-0.5) rewritten as
softmax(0.5*tanh(s/16)) (shift-invariance; sigmoid = affine of tanh, so
tanh+exp share one ACT table set), and the row-sharded output projection
per head extended with a passthrough column carrying the softmax
denominator. Host unshards: divides by the denominators, sums heads and
head-group partials, adds the output bias.

Scores are built transposed ([key, query]) so the PV matmul needs no
transposes and its output lands pre-transposed for the output projection.
"""
import numpy as np
import ml_dtypes
from contextlib import ExitStack

import concourse.bass as bass
import concourse.tile as tile
from concourse import bacc, mybir
from concourse.bass_utils import run_bass_kernel_spmd

BF = ml_dtypes.bfloat16
F32 = mybir.dt.float32
BF16 = mybir.dt.bfloat16

S = 2048          # sequence length
IN = 1024         # input dim
HL = 4            # heads per core
DH = 64           # head dim
KS = IN // 128    # k subtiles of input dim

_CACHE = {}


def _build():
    nc = bacc.Bacc("TRN2", target_bir_lowering=False, debug=False, num_devices=8)
    AF = mybir.ActivationFunctionType

    d_x = [nc.dram_tensor(n, [IN, S], BF16, kind="ExternalInput")
           for n in ("xq", "xk", "xv")]
    d_w = [nc.dram_tensor(n, [IN, 256], BF16, kind="ExternalInput")
           for n in ("wq", "wk", "wv")]
    d_bq = nc.dram_tensor("bq", [128, 2], F32, kind="ExternalInput")
    d_bk = nc.dram_tensor("bk", [128, 2], F32, kind="ExternalInput")
    d_bv = nc.dram_tensor("bv", [1, 256], BF16, kind="ExternalInput")
    d_wo = nc.dram_tensor("wo", [HL, 65, 65], BF16, kind="ExternalInput")
    d_out = nc.dram_tensor("out", [HL, 65, S], F32, kind="ExternalOutput")

    with tile.TileContext(nc) as tc, ExitStack() as ctx:
        const = ctx.enter_context(tc.tile_pool(name="const", bufs=1))
        persist = ctx.enter_context(tc.tile_pool(name="persist", bufs=1))

        # constants
        w_sb = []
        for i in range(3):
            t = const.tile([128, KS, 256], BF16, tag=f"w{i}")
            for ks in range(KS):
                nc.sync.dma_start(t[:, ks, :], d_w[i].ap()[ks * 128:(ks + 1) * 128, :])
            w_sb.append(t)
        bq_sb = const.tile([128, 2], F32, tag="bq")
        nc.sync.dma_start(bq_sb[:], d_bq.ap())
        bk_sb = const.tile([128, 2], F32, tag="bk")
        nc.sync.dma_start(bk_sb[:], d_bk.ap())
        bv_sb = const.tile([1, 256], BF16, tag="bv")
        nc.sync.dma_start(bv_sb[:], d_bv.ap())
        wo_sb = const.tile([65, HL, 65], BF16, tag="wo")
        for h in range(HL):
            nc.sync.dma_start(wo_sb[:, h, :], d_wo.ap()[h])
        ones_row = const.tile([1, 128], BF16, tag="ones")
        nc.vector.memset(ones_row[:], 1.0)

        # persistent activations
        qhT = persist.tile([128, 2, S], BF16, tag="qhT")   # [d'%128, pair, q]
        khT = persist.tile([128, 2, S], BF16, tag="khT")
        vhx = persist.tile([128, 16, HL, 65], BF16, tag="vhx")  # + ones col
        nc.vector.memset(vhx[:, :, :, 64:65], 1.0)

        # ---- Phase A: projections ----
        with tc.tile_pool(name="xpool", bufs=2) as xp, \
             tc.tile_pool(name="pps", bufs=2, space="PSUM") as pps:
            for t_i in range(3):
                x_sb = xp.tile([128, KS, S], BF16, tag="x")
                for ks in range(KS):
                    nc.sync.dma_start(x_sb[:, ks, :],
                                      d_x[t_i].ap()[ks * 128:(ks + 1) * 128, :])
                if t_i < 2:  # q or k -> transposed head layout
                    dest = qhT if t_i == 0 else khT
                    b_sb = bq_sb if t_i == 0 else bk_sb
                    for mb in range(2):
                        for qc in range(4):
                            ps = pps.tile([128, 512], F32, tag="p")
                            for ks in range(KS):
                                nc.tensor.matmul(
                                    ps[:],
                                    w_sb[t_i][:, ks, mb * 128:(mb + 1) * 128],
                                    x_sb[:, ks, qc * 512:(qc + 1) * 512],
                                    start=(ks == 0), stop=(ks == KS - 1))
                            nc.vector.tensor_scalar(
                                dest[:, mb, qc * 512:(qc + 1) * 512], ps[:],
                                b_sb[:, mb:mb + 1], None, mybir.AluOpType.add)
                else:  # v -> natural layout, + bias via K=1 matmul
                    for sb_i in range(16):
                        ps = pps.tile([128, HL, 64], F32, tag="pv")
                        for ks in range(KS):
                            nc.tensor.matmul(
                                ps[:],
                                x_sb[:, ks, sb_i * 128:(sb_i + 1) * 128],
                                w_sb[2][:, ks, :],
                                start=(ks == 0), stop=False)
                        nc.tensor.matmul(ps[:], ones_row[:], bv_sb[:],
                                         start=False, stop=True)
                        nc.vector.tensor_copy(vhx[:, sb_i, :, 0:64], ps[:])

        # ---- Phase B: attention per head ----
        LAG = 2
        with tc.tile_pool(name="wp", bufs=1) as wp, \
             tc.tile_pool(name="thp", bufs=3) as thp, \
             tc.tile_pool(name="outp", bufs=2) as outp, \
             tc.tile_pool(name="qkps", bufs=1, space="PSUM") as qkps, \
             tc.tile_pool(name="pvps", bufs=2, space="PSUM") as pvps:
            for h in range(HL):
                pr, off = h // 2, 64 * (h % 2)
                w_t = wp.tile([128, 16, S], BF16, tag="w")
                pv_t = [None, None]

                def emit_pv(kb, h=h, w_t=w_t, pv_t=pv_t):
                    for qc in range(2):
                        for j in range(2):
                            nc.tensor.matmul(
                                pv_t[qc][0:65, j * 512:(j + 1) * 512],
                                vhx[:, kb, h, :],
                                w_t[:, kb, qc * 1024 + j * 512:
                                    qc * 1024 + (j + 1) * 512],
                                start=(kb == 0), stop=(kb == 15))

                for kb in range(16):
                    qk = qkps.tile([128, S], F32, tag="qk")
                    for hf in range(4):
                        nc.tensor.matmul(
                            qk[:, hf * 512:(hf + 1) * 512],
                            khT[off:off + 64, pr, kb * 128:(kb + 1) * 128],
                            qhT[off:off + 64, pr, hf * 512:(hf + 1) * 512],
                            start=True, stop=True)
                    th = thp.tile([128, S], BF16, tag="th")
                    nc.scalar.activation(th[:], qk[:], AF.Tanh,
                                         bias=0.0, scale=1.0 / 16.0)
                    nc.scalar.activation(w_t[:, kb, :], th[:], AF.Exp,
                                         bias=0.0, scale=0.5)
                    if kb == 0:
                        pv_t[0] = pvps.tile([128, 1024], F32, tag="pv", name=f"pv0_{h}")
                        pv_t[1] = pvps.tile([128, 1024], F32, tag="pv", name=f"pv1_{h}")
                    if kb >= LAG:
                        emit_pv(kb - LAG)
                for kb in range(16 - LAG, 16):
                    emit_pv(kb)
                # output projection per head (row 64 passes softmax sums)
                for qc in range(2):
                    pv_sb = outp.tile([65, 1024], BF16, tag="pvsb")
                    nc.vector.tensor_copy(pv_sb[:], pv_t[qc][0:65, :])
                    y_ps = pvps.tile([128, 1024], F32, tag="pv")
                    for j in range(2):
                        nc.tensor.matmul(y_ps[0:65, j * 512:(j + 1) * 512],
                                         wo_sb[:, h, :],
                                         pv_sb[:, j * 512:(j + 1) * 512],
                                         start=True, stop=True)
                    y_sb = outp.tile([65, 1024], F32, tag="ysb")
                    nc.vector.tensor_copy(y_sb[:], y_ps[0:65, :])
                    nc.sync.dma_start(
                        d_out.ap()[h, :, qc * 1024:(qc + 1) * 1024], y_sb[:])
    nc.compile()
    return nc


def get_module():
    if "nc" not in _CACHE:
        _CACHE["nc"] = _build()
    return _CACHE["nc"]


def make_in_maps(q, k, v, Wq, bq, Wk, bk, Wv, bv, Wo, bo):
    in_maps = []
    for c in range(8):
        b, hg = c // 4, c % 4
        sl = slice(256 * hg, 256 * (hg + 1))
        wo_ext = np.zeros((HL, 65, 65), np.float32)
        for i in range(HL):
            r0 = 256 * hg + 64 * i
            wo_ext[i, 0:64, 0:64] = Wo[r0:r0 + 64, :]
            wo_ext[i, 64, 64] = 1.0
        in_maps.append({
            "xq": np.ascontiguousarray(q[b].T).astype(BF),
            "xk": np.ascontiguousarray(k[b].T).astype(BF),
            "xv": np.ascontiguousarray(v[b].T).astype(BF),
            "wq": np.ascontiguousarray(Wq[:, sl]).astype(BF),
            "wk": np.ascontiguousarray(Wk[:, sl]).astype(BF),
            "wv": np.ascontiguousarray(Wv[:, sl]).astype(BF),
            "bq": np.ascontiguousarray(bq[sl].reshape(2, 128).T).astype(np.float32),
            "bk": np.ascontiguousarray(bk[sl].reshape(2, 128).T).astype(np.float32),
            "bv": bv[sl].reshape(1, 256).astype(BF),
            "wo": wo_ext.astype(BF),
        })
    return in_maps


def assemble(results, bo):
    out = np.zeros((2, S, 64), np.float32)
    for c in range(8):
        y = np.asarray(results[c]["out"], np.float32)   # [4, 65, S]
        acc = (y[:, 0:64, :] / y[:, 64:65, :]).sum(axis=0)  # [64, S]
        out[c // 4] += acc.T
    out += np.asarray(bo, np.float32)[None, None, :]
    return out


def kernel(q, k, v, Wq, bq, Wk, bk, Wv, bv, Wo, bo):
    nc = get_module()
    in_maps = make_in_maps(q, k, v, Wq, bq, Wk, bk, Wv, bv, Wo, bo)
    res = run_bass_kernel_spmd(nc, in_maps, core_ids=list(range(8)))
    return assemble(res.results, bo)


# revision 4
# speedup vs baseline: 33.8214x; 33.8214x over previous
"""MHSA Trainium2 Bass kernel (8 NeuronCores, SPMD).

Sharding: core c handles batch b=c//4, head group hg=c%4 (4 of 16 heads).
Device computes, per core: Q/K/V projections (column-sharded per head
group), head-local attention with softmax(sigmoid(s)-0.5) rewritten as
softmax(0.5*tanh(s/16)) (shift-invariance; sigmoid = affine of tanh, so
tanh+exp share one ACT table set), and the row-sharded output projection
per head extended with a passthrough column carrying the softmax
denominator. Host unshards: divides by the denominators, sums heads and
head-group partials, adds the output bias.

Scores are built transposed ([key, query]) so the PV matmul needs no
transposes and its output lands pre-transposed for the output projection.
"""
import numpy as np
import ml_dtypes
from contextlib import ExitStack

import concourse.bass as bass
import concourse.tile as tile
from concourse import bacc, mybir
from concourse.bass_utils import run_bass_kernel_spmd

BF = ml_dtypes.bfloat16
F32 = mybir.dt.float32
BF16 = mybir.dt.bfloat16

S = 2048          # sequence length
IN = 1024         # input dim
HL = 4            # heads per core
DH = 64           # head dim
KS = IN // 128    # k subtiles of input dim

_CACHE = {}


def _build():
    nc = bacc.Bacc("TRN2", target_bir_lowering=False, debug=False, num_devices=8)
    AF = mybir.ActivationFunctionType

    d_x = [nc.dram_tensor(n, [IN, S], BF16, kind="ExternalInput")
           for n in ("xq", "xk", "xv")]
    d_w = [nc.dram_tensor(n, [IN, 256], BF16, kind="ExternalInput")
           for n in ("wq", "wk", "wv")]
    d_bq = nc.dram_tensor("bq", [128, 2], F32, kind="ExternalInput")
    d_bk = nc.dram_tensor("bk", [128, 2], F32, kind="ExternalInput")
    d_bv = nc.dram_tensor("bv", [1, 256], BF16, kind="ExternalInput")
    d_wo = nc.dram_tensor("wo", [HL, 65, 65], BF16, kind="ExternalInput")
    d_out = nc.dram_tensor("out", [HL, 65, S], F32, kind="ExternalOutput")

    with tile.TileContext(nc) as tc, ExitStack() as ctx:
        const = ctx.enter_context(tc.tile_pool(name="const", bufs=1))
        persist = ctx.enter_context(tc.tile_pool(name="persist", bufs=1))

        # constants
        w_sb = []
        for i in range(3):
            t = const.tile([128, KS, 256], BF16, tag=f"w{i}")
            for ks in range(KS):
                nc.sync.dma_start(t[:, ks, :], d_w[i].ap()[ks * 128:(ks + 1) * 128, :])
            w_sb.append(t)
        bq_sb = const.tile([128, 2], F32, tag="bq")
        nc.sync.dma_start(bq_sb[:], d_bq.ap())
        bk_sb = const.tile([128, 2], F32, tag="bk")
        nc.sync.dma_start(bk_sb[:], d_bk.ap())
        bv_sb = const.tile([1, 256], BF16, tag="bv")
        nc.sync.dma_start(bv_sb[:], d_bv.ap())
        wo_sb = const.tile([65, HL, 65], BF16, tag="wo")
        for h in range(HL):
            nc.sync.dma_start(wo_sb[:, h, :], d_wo.ap()[h])
        ones_row = const.tile([1, 128], BF16, tag="ones")
        nc.vector.memset(ones_row[:], 1.0)

        # persistent activations
        qhT = persist.tile([128, 2, S], BF16, tag="qhT")   # [d'%128, pair, q]
        khT = persist.tile([128, 2, S], BF16, tag="khT")
        vhx = persist.tile([128, 16, HL, 65], BF16, tag="vhx")  # + ones col
        nc.vector.memset(vhx[:, :, :, 64:65], 1.0)

        # ---- Phase A: projections ----
        with tc.tile_pool(name="xpool", bufs=2) as xp, \
             tc.tile_pool(name="pps", bufs=2, space="PSUM") as pps:
            for t_i in range(3):
                x_sb = xp.tile([128, KS, S], BF16, tag="x")
                for ks in range(KS):
                    nc.sync.dma_start(x_sb[:, ks, :],
                                      d_x[t_i].ap()[ks * 128:(ks + 1) * 128, :])
                if t_i < 2:  # q or k -> transposed head layout
                    dest = qhT if t_i == 0 else khT
                    b_sb = bq_sb if t_i == 0 else bk_sb
                    for mb in range(2):
                        for qc in range(4):
                            ps = pps.tile([128, 512], F32, tag="p")
                            for ks in range(KS):
                                nc.tensor.matmul(
                                    ps[:],
                                    w_sb[t_i][:, ks, mb * 128:(mb + 1) * 128],
                                    x_sb[:, ks, qc * 512:(qc + 1) * 512],
                                    start=(ks == 0), stop=(ks == KS - 1))
                            nc.vector.tensor_scalar(
                                dest[:, mb, qc * 512:(qc + 1) * 512], ps[:],
                                b_sb[:, mb:mb + 1], None, mybir.AluOpType.add)
                else:  # v -> natural layout, + bias via K=1 matmul
                    for sb_i in range(16):
                        ps = pps.tile([128, HL, 64], F32, tag="pv")
                        for ks in range(KS):
                            nc.tensor.matmul(
                                ps[:],
                                x_sb[:, ks, sb_i * 128:(sb_i + 1) * 128],
                                w_sb[2][:, ks, :],
                                start=(ks == 0), stop=False)
                        nc.tensor.matmul(ps[:], ones_row[:], bv_sb[:],
                                         start=False, stop=True)
                        nc.vector.tensor_copy(vhx[:, sb_i, :, 0:64], ps[:])

        # ---- Phase B: attention per head ----
        LAG = 2
        with tc.tile_pool(name="wp", bufs=1) as wp, \
             tc.tile_pool(name="thp", bufs=3) as thp, \
             tc.tile_pool(name="outp", bufs=2) as outp, \
             tc.tile_pool(name="qkps", bufs=1, space="PSUM") as qkps, \
             tc.tile_pool(name="pvps", bufs=2, space="PSUM") as pvps:
            for h in range(HL):
                pr, off = h // 2, 64 * (h % 2)
                w_t = wp.tile([128, 16, S], BF16, tag="w")
                pv_t = [None, None]

                def emit_pv(kb, h=h, w_t=w_t, pv_t=pv_t):
                    for qc in range(2):
                        for j in range(2):
                            nc.tensor.matmul(
                                pv_t[qc][0:65, j * 512:(j + 1) * 512],
                                vhx[:, kb, h, :],
                                w_t[:, kb, qc * 1024 + j * 512:
                                    qc * 1024 + (j + 1) * 512],
                                start=(kb == 0), stop=(kb == 15))

                for kb in range(16):
                    qk = qkps.tile([128, S], F32, tag="qk")
                    for hf in range(4):
                        nc.tensor.matmul(
                            qk[:, hf * 512:(hf + 1) * 512],
                            khT[off:off + 64, pr, kb * 128:(kb + 1) * 128],
                            qhT[off:off + 64, pr, hf * 512:(hf + 1) * 512],
                            start=True, stop=True)
                    th = thp.tile([128, S], BF16, tag="th")
                    nc.scalar.activation(th[:], qk[:], AF.Tanh,
                                         bias=0.0, scale=1.0 / 16.0)
                    if kb % 2 == 0:
                        nc.scalar.activation(w_t[:, kb, :], th[:], AF.Exp,
                                             bias=0.0, scale=0.5)
                    else:
                        # DVE cubic: e^(t/2) ~= 1+(t/2)(1+(t/2)(1/2+t/12))
                        s1 = thp.tile([128, S], BF16, tag="scr1")
                        s2 = thp.tile([128, S], BF16, tag="scr2")
                        nc.vector.tensor_scalar(
                            s1[:], th[:], 1.0 / 12.0, 0.5,
                            mybir.AluOpType.mult, mybir.AluOpType.add)
                        nc.vector.tensor_tensor(
                            s2[:], s1[:], th[:], mybir.AluOpType.mult)
                        nc.vector.tensor_scalar(
                            s1[:], s2[:], 0.5, 1.0,
                            mybir.AluOpType.mult, mybir.AluOpType.add)
                        nc.vector.tensor_tensor(
                            s2[:], s1[:], th[:], mybir.AluOpType.mult)
                        nc.vector.tensor_scalar(
                            w_t[:, kb, :], s2[:], 0.5, 1.0,
                            mybir.AluOpType.mult, mybir.AluOpType.add)
                    if kb == 0:
                        pv_t[0] = pvps.tile([128, 1024], F32, tag="pv", name=f"pv0_{h}")
                        pv_t[1] = pvps.tile([128, 1024], F32, tag="pv", name=f"pv1_{h}")
                    if kb >= LAG:
                        emit_pv(kb - LAG)
                for kb in range(16 - LAG, 16):
                    emit_pv(kb)
                # output projection per head (row 64 passes softmax sums)
                for qc in range(2):
                    pv_sb = outp.tile([65, 1024], BF16, tag="pvsb")
                    nc.vector.tensor_copy(pv_sb[:], pv_t[qc][0:65, :])
                    y_ps = pvps.tile([128, 1024], F32, tag="pv")
                    for j in range(2):
                        nc.tensor.matmul(y_ps[0:65, j * 512:(j + 1) * 512],
                                         wo_sb[:, h, :],
                                         pv_sb[:, j * 512:(j + 1) * 512],
                                         start=True, stop=True)
                    y_sb = outp.tile([65, 1024], F32, tag="ysb")
                    nc.vector.tensor_copy(y_sb[:], y_ps[0:65, :])
                    nc.sync.dma_start(
                        d_out.ap()[h, :, qc * 1024:(qc + 1) * 1024], y_sb[:])
    nc.compile()
    return nc


def get_module():
    if "nc" not in _CACHE:
        _CACHE["nc"] = _build()
    return _CACHE["nc"]


def make_in_maps(q, k, v, Wq, bq, Wk, bk, Wv, bv, Wo, bo):
    in_maps = []
    for c in range(8):
        b, hg = c // 4, c % 4
        sl = slice(256 * hg, 256 * (hg + 1))
        wo_ext = np.zeros((HL, 65, 65), np.float32)
        for i in range(HL):
            r0 = 256 * hg + 64 * i
            wo_ext[i, 0:64, 0:64] = Wo[r0:r0 + 64, :]
            wo_ext[i, 64, 64] = 1.0
        in_maps.append({
            "xq": np.ascontiguousarray(q[b].T).astype(BF),
            "xk": np.ascontiguousarray(k[b].T).astype(BF),
            "xv": np.ascontiguousarray(v[b].T).astype(BF),
            "wq": np.ascontiguousarray(Wq[:, sl]).astype(BF),
            "wk": np.ascontiguousarray(Wk[:, sl]).astype(BF),
            "wv": np.ascontiguousarray(Wv[:, sl]).astype(BF),
            "bq": np.ascontiguousarray(bq[sl].reshape(2, 128).T).astype(np.float32),
            "bk": np.ascontiguousarray(bk[sl].reshape(2, 128).T).astype(np.float32),
            "bv": bv[sl].reshape(1, 256).astype(BF),
            "wo": wo_ext.astype(BF),
        })
    return in_maps


def assemble(results, bo):
    out = np.zeros((2, S, 64), np.float32)
    for c in range(8):
        y = np.asarray(results[c]["out"], np.float32)   # [4, 65, S]
        acc = (y[:, 0:64, :] / y[:, 64:65, :]).sum(axis=0)  # [64, S]
        out[c // 4] += acc.T
    out += np.asarray(bo, np.float32)[None, None, :]
    return out


def kernel(q, k, v, Wq, bq, Wk, bk, Wv, bv, Wo, bo):
    nc = get_module()
    in_maps = make_in_maps(q, k, v, Wq, bq, Wk, bk, Wv, bv, Wo, bo)
    res = run_bass_kernel_spmd(nc, in_maps, core_ids=list(range(8)))
    return assemble(res.results, bo)


# revision 5
# speedup vs baseline: 164.9854x; 4.8781x over previous
"""MHSA Trainium2 Bass kernel (8 NeuronCores, SPMD).

Sharding: core c handles batch b=c//4, head group hg=c%4 (4 of 16 heads).
Device computes, per core: Q/K/V projections (column-sharded per head
group), head-local attention with softmax(sigmoid(s)-0.5) rewritten as
softmax(0.5*tanh(s/16)) (shift-invariance; sigmoid = affine of tanh, so
tanh+exp share one ACT table set), and the row-sharded output projection
per head extended with a passthrough column carrying the softmax
denominator. Host unshards: divides by the denominators, sums heads and
head-group partials, adds the output bias.

Scores are built transposed ([key, query]) so the PV matmul needs no
transposes and its output lands pre-transposed for the output projection.
"""
import numpy as np
import ml_dtypes
from contextlib import ExitStack

import concourse.bass as bass
import concourse.tile as tile
from concourse import bacc, mybir
from concourse.bass_utils import run_bass_kernel_spmd

BF = ml_dtypes.bfloat16
F32 = mybir.dt.float32
BF16 = mybir.dt.bfloat16

S = 2048          # sequence length
IN = 1024         # input dim
HL = 4            # heads per core
DH = 64           # head dim
KS = IN // 128    # k subtiles of input dim

_CACHE = {}


def _build():
    nc = bacc.Bacc("TRN2", target_bir_lowering=False, debug=False, num_devices=8)
    AF = mybir.ActivationFunctionType

    d_x = [nc.dram_tensor(n, [IN, S], BF16, kind="ExternalInput")
           for n in ("xq", "xk", "xv")]
    d_w = [nc.dram_tensor(n, [IN, 256], BF16, kind="ExternalInput")
           for n in ("wq", "wk", "wv")]
    d_bq = nc.dram_tensor("bq", [128, 2], F32, kind="ExternalInput")
    d_bk = nc.dram_tensor("bk", [128, 2], F32, kind="ExternalInput")
    d_bv = nc.dram_tensor("bv", [1, 256], BF16, kind="ExternalInput")
    d_wo = nc.dram_tensor("wo", [HL, 65, 65], BF16, kind="ExternalInput")
    d_out = nc.dram_tensor("out", [HL, 65, S], F32, kind="ExternalOutput")

    with tile.TileContext(nc) as tc, ExitStack() as ctx:
        const = ctx.enter_context(tc.tile_pool(name="const", bufs=1))
        persist = ctx.enter_context(tc.tile_pool(name="persist", bufs=1))

        # constants
        w_sb = []
        for i in range(3):
            t = const.tile([128, KS, 256], BF16, tag=f"w{i}")
            for ks in range(KS):
                nc.sync.dma_start(t[:, ks, :], d_w[i].ap()[ks * 128:(ks + 1) * 128, :])
            w_sb.append(t)
        bq_sb = const.tile([128, 2], F32, tag="bq")
        nc.sync.dma_start(bq_sb[:], d_bq.ap())
        bk_sb = const.tile([128, 2], F32, tag="bk")
        nc.sync.dma_start(bk_sb[:], d_bk.ap())
        bv_sb = const.tile([1, 256], BF16, tag="bv")
        nc.sync.dma_start(bv_sb[:], d_bv.ap())
        wo_sb = const.tile([65, HL, 65], BF16, tag="wo")
        for h in range(HL):
            nc.sync.dma_start(wo_sb[:, h, :], d_wo.ap()[h])
        ones_row = const.tile([1, 128], BF16, tag="ones")
        nc.vector.memset(ones_row[:], 1.0)

        # persistent activations
        qhT = persist.tile([128, 2, S], BF16, tag="qhT")   # [d'%128, pair, q]
        khT = persist.tile([128, 2, S], BF16, tag="khT")
        vhx = persist.tile([128, 16, HL, 65], BF16, tag="vhx")  # + ones col
        nc.vector.memset(vhx[:, :, :, 64:65], 1.0)

        # ---- Phase A: projections ----
        with tc.tile_pool(name="xpool", bufs=2) as xp, \
             tc.tile_pool(name="pps", bufs=2, space="PSUM") as pps:
            for t_i in (2, 0, 1):  # v first: attention only waits on q/k
                x_sb = xp.tile([128, KS, S], BF16, tag="x")
                for ks in range(KS):
                    nc.sync.dma_start(x_sb[:, ks, :],
                                      d_x[t_i].ap()[ks * 128:(ks + 1) * 128, :])
                if t_i < 2:  # q or k -> transposed head layout
                    dest = qhT if t_i == 0 else khT
                    b_sb = bq_sb if t_i == 0 else bk_sb
                    for mb in range(2):
                        for qc in range(4):
                            ps = pps.tile([128, 512], F32, tag="p")
                            for ks in range(KS):
                                nc.tensor.matmul(
                                    ps[:],
                                    w_sb[t_i][:, ks, mb * 128:(mb + 1) * 128],
                                    x_sb[:, ks, qc * 512:(qc + 1) * 512],
                                    start=(ks == 0), stop=(ks == KS - 1))
                            nc.vector.tensor_scalar(
                                dest[:, mb, qc * 512:(qc + 1) * 512], ps[:],
                                b_sb[:, mb:mb + 1], None, mybir.AluOpType.add)
                else:  # v -> natural layout, + bias via K=1 matmul
                    for sb_i in range(16):
                        ps = pps.tile([128, HL, 64], F32, tag="pv")
                        for ks in range(KS):
                            nc.tensor.matmul(
                                ps[:],
                                x_sb[:, ks, sb_i * 128:(sb_i + 1) * 128],
                                w_sb[2][:, ks, :],
                                start=(ks == 0), stop=False)
                        nc.tensor.matmul(ps[:], ones_row[:], bv_sb[:],
                                         start=False, stop=True)
                        nc.vector.tensor_copy(vhx[:, sb_i, :, 0:64], ps[:])

        # ---- Phase B: attention per head ----
        LAG = 2
        with tc.tile_pool(name="wp", bufs=1) as wp, \
             tc.tile_pool(name="thp", bufs=3) as thp, \
             tc.tile_pool(name="outp", bufs=2) as outp, \
             tc.tile_pool(name="qkps", bufs=1, space="PSUM") as qkps, \
             tc.tile_pool(name="pvps", bufs=2, space="PSUM") as pvps:
            for h in range(HL):
                pr, off = h // 2, 64 * (h % 2)
                w_t = wp.tile([128, 16, S], BF16, tag="w")
                pv_t = [None, None]

                def emit_pv(kb, h=h, w_t=w_t, pv_t=pv_t):
                    for qc in range(2):
                        for j in range(2):
                            nc.tensor.matmul(
                                pv_t[qc][0:65, j * 512:(j + 1) * 512],
                                vhx[:, kb, h, :],
                                w_t[:, kb, qc * 1024 + j * 512:
                                    qc * 1024 + (j + 1) * 512],
                                start=(kb == 0), stop=(kb == 15))

                for kb in range(16):
                    qk = qkps.tile([128, S], F32, tag="qk")
                    for hf in range(4):
                        nc.tensor.matmul(
                            qk[:, hf * 512:(hf + 1) * 512],
                            khT[off:off + 64, pr, kb * 128:(kb + 1) * 128],
                            qhT[off:off + 64, pr, hf * 512:(hf + 1) * 512],
                            start=True, stop=True)
                    th = thp.tile([128, S], BF16, tag="th")
                    nc.scalar.activation(th[:], qk[:], AF.Tanh,
                                         bias=0.0, scale=1.0 / 16.0)
                    if kb % 2 == 0:
                        nc.scalar.activation(w_t[:, kb, :], th[:], AF.Exp,
                                             bias=0.0, scale=0.5)
                    else:
                        # DVE cubic: e^(t/2) ~= 1+(t/2)(1+(t/2)(1/2+t/12))
                        s1 = thp.tile([128, S], BF16, tag="scr1")
                        s2 = thp.tile([128, S], BF16, tag="scr2")
                        nc.vector.tensor_scalar(
                            s1[:], th[:], 1.0 / 12.0, 0.5,
                            mybir.AluOpType.mult, mybir.AluOpType.add)
                        nc.vector.tensor_tensor(
                            s2[:], s1[:], th[:], mybir.AluOpType.mult)
                        nc.vector.tensor_scalar(
                            s1[:], s2[:], 0.5, 1.0,
                            mybir.AluOpType.mult, mybir.AluOpType.add)
                        nc.vector.tensor_tensor(
                            s2[:], s1[:], th[:], mybir.AluOpType.mult)
                        nc.vector.tensor_scalar(
                            w_t[:, kb, :], s2[:], 0.5, 1.0,
                            mybir.AluOpType.mult, mybir.AluOpType.add)
                    if kb == 0:
                        pv_t[0] = pvps.tile([128, 1024], F32, tag="pv", name=f"pv0_{h}")
                        pv_t[1] = pvps.tile([128, 1024], F32, tag="pv", name=f"pv1_{h}")
                    if kb >= LAG:
                        emit_pv(kb - LAG)
                for kb in range(16 - LAG, 16):
                    emit_pv(kb)
                # output projection per head (row 64 passes softmax sums)
                for qc in range(2):
                    pv_sb = outp.tile([65, 1024], BF16, tag="pvsb")
                    nc.vector.tensor_copy(pv_sb[:], pv_t[qc][0:65, :])
                    y_ps = pvps.tile([128, 1024], F32, tag="pv")
                    for j in range(2):
                        nc.tensor.matmul(y_ps[0:65, j * 512:(j + 1) * 512],
                                         wo_sb[:, h, :],
                                         pv_sb[:, j * 512:(j + 1) * 512],
                                         start=True, stop=True)
                    y_sb = outp.tile([65, 1024], F32, tag="ysb")
                    nc.vector.tensor_copy(y_sb[:], y_ps[0:65, :])
                    nc.sync.dma_start(
                        d_out.ap()[h, :, qc * 1024:(qc + 1) * 1024], y_sb[:])
    nc.compile()
    return nc


def get_module():
    if "nc" not in _CACHE:
        _CACHE["nc"] = _build()
    return _CACHE["nc"]


def make_in_maps(q, k, v, Wq, bq, Wk, bk, Wv, bv, Wo, bo):
    in_maps = []
    for c in range(8):
        b, hg = c // 4, c % 4
        sl = slice(256 * hg, 256 * (hg + 1))
        wo_ext = np.zeros((HL, 65, 65), np.float32)
        for i in range(HL):
            r0 = 256 * hg + 64 * i
            wo_ext[i, 0:64, 0:64] = Wo[r0:r0 + 64, :]
            wo_ext[i, 64, 64] = 1.0
        in_maps.append({
            "xq": np.ascontiguousarray(q[b].T).astype(BF),
            "xk": np.ascontiguousarray(k[b].T).astype(BF),
            "xv": np.ascontiguousarray(v[b].T).astype(BF),
            "wq": np.ascontiguousarray(Wq[:, sl]).astype(BF),
            "wk": np.ascontiguousarray(Wk[:, sl]).astype(BF),
            "wv": np.ascontiguousarray(Wv[:, sl]).astype(BF),
            "bq": np.ascontiguousarray(bq[sl].reshape(2, 128).T).astype(np.float32),
            "bk": np.ascontiguousarray(bk[sl].reshape(2, 128).T).astype(np.float32),
            "bv": bv[sl].reshape(1, 256).astype(BF),
            "wo": wo_ext.astype(BF),
        })
    return in_maps


def assemble(results, bo):
    out = np.zeros((2, S, 64), np.float32)
    for c in range(8):
        y = np.asarray(results[c]["out"], np.float32)   # [4, 65, S]
        acc = (y[:, 0:64, :] / y[:, 64:65, :]).sum(axis=0)  # [64, S]
        out[c // 4] += acc.T
    out += np.asarray(bo, np.float32)[None, None, :]
    return out


def kernel(q, k, v, Wq, bq, Wk, bk, Wv, bv, Wo, bo):
    nc = get_module()
    in_maps = make_in_maps(q, k, v, Wq, bq, Wk, bk, Wv, bv, Wo, bo)
    res = run_bass_kernel_spmd(nc, in_maps, core_ids=list(range(8)))
    return assemble(res.results, bo)


# revision 6
# speedup vs baseline: 13473.7110x; 81.6661x over previous
"""MHSA Trainium2 Bass kernel (8 NeuronCores, SPMD).

Sharding: core c handles batch b=c//4, head group hg=c%4 (4 of 16 heads).
Device computes, per core: Q/K/V projections (column-sharded per head
group), head-local attention with softmax(sigmoid(s)-0.5) rewritten as
softmax(0.5*tanh(s/16)) (shift-invariance; sigmoid = affine of tanh, so
tanh+exp share one ACT table set), and the row-sharded output projection
per head extended with a passthrough column carrying the softmax
denominator. Host unshards: divides by the denominators, sums heads and
head-group partials, adds the output bias.

Scores are built transposed ([key, query]) so the PV matmul needs no
transposes and its output lands pre-transposed for the output projection.
"""
import numpy as np
import ml_dtypes
from contextlib import ExitStack

import concourse.bass as bass
import concourse.tile as tile
from concourse import bacc, mybir
from concourse.bass_utils import run_bass_kernel_spmd

BF = ml_dtypes.bfloat16
F32 = mybir.dt.float32
BF16 = mybir.dt.bfloat16

S = 2048          # sequence length
IN = 1024         # input dim
HL = 4            # heads per core
DH = 64           # head dim
KS = IN // 128    # k subtiles of input dim

_CACHE = {}


def _build():
    nc = bacc.Bacc("TRN2", target_bir_lowering=False, debug=False, num_devices=8)
    AF = mybir.ActivationFunctionType

    d_x = [nc.dram_tensor(n, [IN, S], BF16, kind="ExternalInput")
           for n in ("xq", "xk", "xv")]
    d_w = [nc.dram_tensor(n, [IN, 256], BF16, kind="ExternalInput")
           for n in ("wq", "wk", "wv")]
    d_bq = nc.dram_tensor("bq", [128, 2], F32, kind="ExternalInput")
    d_bk = nc.dram_tensor("bk", [128, 2], F32, kind="ExternalInput")
    d_bv = nc.dram_tensor("bv", [1, 256], BF16, kind="ExternalInput")
    d_wo = nc.dram_tensor("wo", [HL, 65, 65], BF16, kind="ExternalInput")
    d_out = nc.dram_tensor("out", [HL, 65, S], F32, kind="ExternalOutput")

    with tile.TileContext(nc) as tc, ExitStack() as ctx:
        const = ctx.enter_context(tc.tile_pool(name="const", bufs=1))
        persist = ctx.enter_context(tc.tile_pool(name="persist", bufs=1))

        # constants
        w_sb = []
        for i in range(3):
            t = const.tile([128, KS, 256], BF16, tag=f"w{i}")
            for ks in range(KS):
                nc.sync.dma_start(t[:, ks, :], d_w[i].ap()[ks * 128:(ks + 1) * 128, :])
            w_sb.append(t)
        bq_sb = const.tile([128, 2], F32, tag="bq")
        nc.sync.dma_start(bq_sb[:], d_bq.ap())
        bk_sb = const.tile([128, 2], F32, tag="bk")
        nc.sync.dma_start(bk_sb[:], d_bk.ap())
        bv_sb = const.tile([1, 256], BF16, tag="bv")
        nc.sync.dma_start(bv_sb[:], d_bv.ap())
        wo_sb = const.tile([65, HL, 65], BF16, tag="wo")
        for h in range(HL):
            nc.sync.dma_start(wo_sb[:, h, :], d_wo.ap()[h])
        ones_row = const.tile([1, 128], BF16, tag="ones")
        nc.vector.memset(ones_row[:], 1.0)

        # persistent activations
        qhT = persist.tile([128, 2, S], BF16, tag="qhT")   # [d'%128, pair, q]
        khT = persist.tile([128, 2, S], BF16, tag="khT")
        vhx = persist.tile([128, 16, HL, 65], BF16, tag="vhx")  # + ones col
        nc.vector.memset(vhx[:, :, :, 64:65], 1.0)

        # ---- Phase A: projections ----
        with tc.tile_pool(name="xpool", bufs=2) as xp, \
             tc.tile_pool(name="pps", bufs=2, space="PSUM") as pps:
            for t_i in (2, 0, 1):  # v first: attention only waits on q/k
                x_sb = xp.tile([128, KS, S], BF16, tag="x")
                for ks in range(KS):
                    nc.sync.dma_start(x_sb[:, ks, :],
                                      d_x[t_i].ap()[ks * 128:(ks + 1) * 128, :])
                if t_i < 2:  # q or k -> transposed head layout
                    dest = qhT if t_i == 0 else khT
                    b_sb = bq_sb if t_i == 0 else bk_sb
                    for mb in range(2):
                        for qc in range(4):
                            ps = pps.tile([128, 512], F32, tag="p")
                            for ks in range(KS):
                                nc.tensor.matmul(
                                    ps[:],
                                    w_sb[t_i][:, ks, mb * 128:(mb + 1) * 128],
                                    x_sb[:, ks, qc * 512:(qc + 1) * 512],
                                    start=(ks == 0), stop=(ks == KS - 1))
                            nc.vector.tensor_scalar(
                                dest[:, mb, qc * 512:(qc + 1) * 512], ps[:],
                                b_sb[:, mb:mb + 1], None, mybir.AluOpType.add)
                else:  # v -> natural layout, + bias via K=1 matmul
                    for sb_i in range(16):
                        ps = pps.tile([128, HL, 64], F32, tag="pv")
                        for ks in range(KS):
                            nc.tensor.matmul(
                                ps[:],
                                x_sb[:, ks, sb_i * 128:(sb_i + 1) * 128],
                                w_sb[2][:, ks, :],
                                start=(ks == 0), stop=False)
                        nc.tensor.matmul(ps[:], ones_row[:], bv_sb[:],
                                         start=False, stop=True)
                        nc.vector.tensor_copy(vhx[:, sb_i, :, 0:64], ps[:])

        # ---- Phase B: attention per head ----
        LAG = 2
        with tc.tile_pool(name="wp", bufs=1) as wp, \
             tc.tile_pool(name="thp", bufs=3) as thp, \
             tc.tile_pool(name="outp", bufs=2) as outp, \
             tc.tile_pool(name="qkps", bufs=1, space="PSUM") as qkps, \
             tc.tile_pool(name="pvps", bufs=2, space="PSUM") as pvps:
            for h in range(HL):
                pr, off = h // 2, 64 * (h % 2)
                w_t = wp.tile([128, 16, S], BF16, tag="w")
                pv_t = [None, None]

                def emit_pv(kb, h=h, w_t=w_t, pv_t=pv_t):
                    for qc in range(2):
                        for j in range(2):
                            nc.tensor.matmul(
                                pv_t[qc][0:65, j * 512:(j + 1) * 512],
                                vhx[:, kb, h, :],
                                w_t[:, kb, qc * 1024 + j * 512:
                                    qc * 1024 + (j + 1) * 512],
                                start=(kb == 0), stop=(kb == 15))

                for kb in range(16):
                    qk = qkps.tile([128, S], F32, tag="qk")
                    for hf in range(4):
                        nc.tensor.matmul(
                            qk[:, hf * 512:(hf + 1) * 512],
                            khT[off:off + 64, pr, kb * 128:(kb + 1) * 128],
                            qhT[off:off + 64, pr, hf * 512:(hf + 1) * 512],
                            start=True, stop=True)
                    th = thp.tile([128, S], BF16, tag="th")
                    nc.scalar.activation(th[:], qk[:], AF.Tanh,
                                         bias=0.0, scale=1.0 / 16.0)
                    if kb % 2 == 0 and kb < 14:  # 7 tiles on ACT, 9 on DVE
                        nc.scalar.activation(w_t[:, kb, :], th[:], AF.Exp,
                                             bias=0.0, scale=0.5)
                    else:
                        # DVE, 4 ops: e^(t/2) = (p(v))^2, v=t/4, p minimax
                        # quadratic 1 + c1*v + v^2/2 on |v|<=1/4
                        s1 = thp.tile([128, S], BF16, tag="scr1")
                        s2 = thp.tile([128, S], BF16, tag="scr2")
                        nc.vector.tensor_scalar(
                            s1[:], th[:], 1.0 / 8.0, 1.0078125,
                            mybir.AluOpType.mult, mybir.AluOpType.add)
                        nc.vector.tensor_tensor(
                            s2[:], s1[:], th[:], mybir.AluOpType.mult)
                        nc.vector.tensor_scalar(
                            s1[:], s2[:], 0.25, 1.0,
                            mybir.AluOpType.mult, mybir.AluOpType.add)
                        nc.vector.tensor_tensor(
                            w_t[:, kb, :], s1[:], s1[:], mybir.AluOpType.mult)
                    if kb == 0:
                        pv_t[0] = pvps.tile([128, 1024], F32, tag="pv", name=f"pv0_{h}")
                        pv_t[1] = pvps.tile([128, 1024], F32, tag="pv", name=f"pv1_{h}")
                    if kb >= LAG:
                        emit_pv(kb - LAG)
                for kb in range(16 - LAG, 16):
                    emit_pv(kb)
                # output projection per head (row 64 passes softmax sums)
                for qc in range(2):
                    pv_sb = outp.tile([65, 1024], BF16, tag="pvsb")
                    nc.vector.tensor_copy(pv_sb[:], pv_t[qc][0:65, :])
                    y_ps = pvps.tile([128, 1024], F32, tag="pv")
                    for j in range(2):
                        nc.tensor.matmul(y_ps[0:65, j * 512:(j + 1) * 512],
                                         wo_sb[:, h, :],
                                         pv_sb[:, j * 512:(j + 1) * 512],
                                         start=True, stop=True)
                    y_sb = outp.tile([65, 1024], F32, tag="ysb")
                    nc.vector.tensor_copy(y_sb[:], y_ps[0:65, :])
                    nc.sync.dma_start(
                        d_out.ap()[h, :, qc * 1024:(qc + 1) * 1024], y_sb[:])
    nc.compile()
    return nc


def get_module():
    if "nc" not in _CACHE:
        _CACHE["nc"] = _build()
    return _CACHE["nc"]


def make_in_maps(q, k, v, Wq, bq, Wk, bk, Wv, bv, Wo, bo):
    in_maps = []
    for c in range(8):
        b, hg = c // 4, c % 4
        sl = slice(256 * hg, 256 * (hg + 1))
        wo_ext = np.zeros((HL, 65, 65), np.float32)
        for i in range(HL):
            r0 = 256 * hg + 64 * i
            wo_ext[i, 0:64, 0:64] = Wo[r0:r0 + 64, :]
            wo_ext[i, 64, 64] = 1.0
        in_maps.append({
            "xq": np.ascontiguousarray(q[b].T).astype(BF),
            "xk": np.ascontiguousarray(k[b].T).astype(BF),
            "xv": np.ascontiguousarray(v[b].T).astype(BF),
            "wq": np.ascontiguousarray(Wq[:, sl]).astype(BF),
            "wk": np.ascontiguousarray(Wk[:, sl]).astype(BF),
            "wv": np.ascontiguousarray(Wv[:, sl]).astype(BF),
            "bq": np.ascontiguousarray(bq[sl].reshape(2, 128).T).astype(np.float32),
            "bk": np.ascontiguousarray(bk[sl].reshape(2, 128).T).astype(np.float32),
            "bv": bv[sl].reshape(1, 256).astype(BF),
            "wo": wo_ext.astype(BF),
        })
    return in_maps


def assemble(results, bo):
    out = np.zeros((2, S, 64), np.float32)
    for c in range(8):
        y = np.asarray(results[c]["out"], np.float32)   # [4, 65, S]
        acc = (y[:, 0:64, :] / y[:, 64:65, :]).sum(axis=0)  # [64, S]
        out[c // 4] += acc.T
    out += np.asarray(bo, np.float32)[None, None, :]
    return out


def kernel(q, k, v, Wq, bq, Wk, bk, Wv, bv, Wo, bo):
    nc = get_module()
    in_maps = make_in_maps(q, k, v, Wq, bq, Wk, bk, Wv, bv, Wo, bo)
    res = run_bass_kernel_spmd(nc, in_maps, core_ids=list(range(8)))
    return assemble(res.results, bo)
